# revision 36
# baseline (speedup 1.0000x reference)
"""ColBERT MaxSim kernel for 8 Trainium2 NeuronCores.

scores[b, c] = sum_n max_s (qs[b, n, :] . ps[c, s, :])
  qs: (64, 32, 128) f32, ps: (64, 1024, 128) f32 -> scores: (64, 64) f32

Sharding: docs (c) are sharded 8 per core; qs is replicated. Each core
computes its (64, 8) score tile; the host concatenates along c. This puts
only ~2.5 MiB of DMA on each core (vs 33 MiB for batch-sharding) while the
per-core compute volume is identical.

Per-core dataflow (mode "pair", the default):
  - fp32 matmul on TRN2 streams at 2 cyc/col x 2 passes = 4x slower than
    16-bit, so inputs are cast to fp16 on the host (10 mantissa bits;
    measured end-to-end rel err ~2.4e-5 vs the fp32 reference).
  - Doc tokens are combined in PAIRS on the host: P+ = (P_even+P_odd)/2,
    P- = (P_even-P_odd)/2. Using max(a,b) = (a+b)/2 + |a-b|/2:
      PE:  S = Q.P+  (one PSUM bank),  D = Q.P-  (second bank)
      ACT: A = |D|   (pointwise Abs, PSUM->SBUF fp16, the only non-DVE
                      engine that can read PSUM)
      PE:  S += I.A  (accumulating identity matmul, start=False) — S's
                      bank now holds the 512 per-pair maxes
      DVE: reduce_max over [128, 2, 512] (two docs per op) -> maxcols
    This halves the VectorE reduce volume (the kernel's true bottleneck:
    PSUM can only be drained by VectorE/ScalarE at 1 elem/lane/cycle, and
    every reduce op on this silicon runs at 1x regardless of dtype), and
    lands PE/ACT/DVE each at ~650-690 ns per (M-group, doc) tile —
    balanced and ~98% occupied in steady state.
  - Token-sum over each query's 32 rows via one small fp32 matmul with a
    block-diagonal ones matrix: out[4, 128] = onesT.T @ maxcols.

Mode "direct" (env KERNEL_MODE=direct) is the exact-fp32 fallback:
fp32 matmuls + VectorE reduce_max straight from PSUM (~2.6x slower).
"""

import os
import sys
from contextlib import ExitStack

import numpy as np

sys.path.insert(0, "/opt/trn_rl_repo")
sys.path.insert(0, "/opt/trn_rl_repo/concourse")

import bass_rust
import concourse.bass as bass
import concourse.mybir as mybir
import concourse.tile as tile
from concourse import bass_utils

# Problem shape (hardcoded per contract)
N_CORES = 8
NQ, TQ, D = 64, 32, 128          # queries, query tokens, dim
ND, TD = 64, 1024                # docs, doc tokens
DOCS_PER_CORE = ND // N_CORES    # 8
QROWS = NQ * TQ                  # 2048 query-token rows
MG = QROWS // 128                # 16 M-groups of 128 rows
QPG = 128 // TQ                  # 4 queries per M-group
NPAIR = TD // 2                  # 512 token pairs per doc

F32 = mybir.dt.float32
F16 = mybir.dt.float16

MODE = os.environ.get("KERNEL_MODE", "pair")


def _split_multi_waits(nc):
    """This walrus build rejects >1 embedded sync wait per instruction
    ("Too many sync wait commands"). Split extras onto single-wait NoOps
    inserted just before the instruction on the same engine — semantically
    identical (per-engine program order is preserved)."""
    n_split = 0
    for fn in nc.m.functions:
        for blk in fn.blocks:
            out = []
            for ins in blk.instructions:
                si = ins.sync_info
                waits = list(si.on_wait) if si and si.on_wait else []
                if len(waits) > 1:
                    for j, w in enumerate(waits[:-1]):
                        nop = mybir.InstNoOp(
                            name=f"{ins.name}_sw{j}", ins=[], outs=[])
                        nop.engine = ins.engine
                        nop.sync_info = bass_rust.SyncInfo(
                            on_wait=[w], on_update=[])
                        out.append(nop)
                    ins.sync_info = bass_rust.SyncInfo(
                        on_wait=[waits[-1]], on_update=list(si.on_update))
                    n_split += 1
                out.append(ins)
            blk.instructions = out
    return n_split


def _build_pair_module():
    nc = bass.Bass("TRN2", target_bir_lowering=False, debug=False)

    qsT = nc.dram_tensor("qsT", [D, QROWS], F16, kind="ExternalInput").ap()
    psP = nc.dram_tensor("psP", [D, DOCS_PER_CORE * NPAIR], F16,
                         kind="ExternalInput").ap()
    psM = nc.dram_tensor("psM", [D, DOCS_PER_CORE * NPAIR], F16,
                         kind="ExternalInput").ap()
    ident = nc.dram_tensor("ident", [128, 128], F16,
                           kind="ExternalInput").ap()
    ones = nc.dram_tensor("ones", [128, QPG], F32, kind="ExternalInput").ap()
    out = nc.dram_tensor("out", [NQ, DOCS_PER_CORE], F32,
                         kind="ExternalOutput").ap()

    with tile.TileContext(nc) as tc, ExitStack() as ctx:
        const = ctx.enter_context(tc.tile_pool(name="const", bufs=1))
        stage = ctx.enter_context(tc.tile_pool(name="stage", bufs=10))
        psumS = ctx.enter_context(
            tc.tile_pool(name="psumS", bufs=2, space="PSUM"))
        psumD = ctx.enter_context(
            tc.tile_pool(name="psumD", bufs=4, space="PSUM"))

        # DMA issue costs ~650ns per dma_start on a sequencer: consolidate
        # into few chunks and split issues across both HWDGE engines
        # (sync + scalar) so compute starts ASAP. First chunks cover the
        # first two docs only, so the first tiles' deps land quickly.
        qsT_sb = const.tile([D, QROWS], F16)
        psP_sb = const.tile([D, DOCS_PER_CORE * NPAIR], F16)
        psM_sb = const.tile([D, DOCS_PER_CORE * NPAIR], F16)
        ident_sb = const.tile([128, 128], F16)
        ones_sb = const.tile([128, QPG], F32)
        c0 = 2 * NPAIR  # first chunk: docs 0-1
        q0 = 256        # first chunk of qsT: M-groups 0-1 (64 KB)
        nc.sync.dma_start(qsT_sb[:, 0:q0], qsT[:, 0:q0])
        nc.scalar.dma_start(psM_sb[:, 0:c0], psM[:, 0:c0])
        nc.sync.dma_start(psP_sb[:, 0:c0], psP[:, 0:c0])
        # Prefetch the Abs ACT table set (~2.7us TABLE_LOAD + drain) NOW —
        # emitted here it overlaps the initial DMA transfers instead of
        # gating the first real abs (walrus inserts the table load right
        # before the first ACTIVATE in the ScalarE stream).
        warm = stage.tile([1, 2], F16, tag="warm")
        nc.gpsimd.memset(warm[:], 0.0)
        warm2 = stage.tile([1, 2], F16, tag="warm2")
        nc.scalar.activation(warm2[:], warm[:],
                             mybir.ActivationFunctionType.Abs)
        nc.scalar.dma_start(ident_sb[:], ident[:])
        nc.sync.dma_start(qsT_sb[:, q0:], qsT[:, q0:])
        nc.scalar.dma_start(psM_sb[:, c0:], psM[:, c0:])
        nc.sync.dma_start(psP_sb[:, c0:], psP[:, c0:])
        nc.sync.dma_start(ones_sb[:], ones[:])

        # HAM warmup: the PE is otherwise idle from the end of the NEFF
        # preamble (~7.5us) until the first DMA chunks land (~13us), and the
        # HAM clock gate needs ~3.4us of sustained PE activity to lift the
        # throttle from 1.2 to 2.4 GHz. A burst of matmuls on an
        # uninitialized (never-read) tile fills the activity window for
        # free, so the real matmul stream starts warm.
        garbage = const.tile([128, NPAIR], F16)
        nc.gpsimd.memset(garbage[:], 0.0)
        for _ in range(12):
            wt = psumD.tile([128, NPAIR], F32, tag="d")
            nc.tensor.matmul(wt[:], lhsT=garbage[:, 0:128], rhs=garbage[:],
                             start=True, stop=True)
        # Second warmup burst, gated on the first qsT chunk: on cores whose
        # DMA lands late (8-core HBM contention skews arrivals by ~2.5us)
        # the ungated burst ends too early, the PE idles >3.4us and HAM
        # re-throttles right as the real stream starts. These bridge that
        # gap; on fast cores they finish before the ps chunks land anyway.
        for _ in range(6):
            wt = psumD.tile([128, NPAIR], F32, tag="d")
            nc.tensor.matmul(wt[:], lhsT=qsT_sb[:, 0:128],
                             rhs=garbage[:], start=True, stop=True)

        # maxcols[p, mg*8 + dloc] = max over doc dloc's tokens for row p of mg
        maxcols = const.tile([128, MG * DOCS_PER_CORE], F32)

        for dp in range(DOCS_PER_CORE // 2):
            for mg in range(MG):
                lhsT = qsT_sb[:, mg * 128:(mg + 1) * 128]
                # Two docs (2*dp, 2*dp+1) share one 2-bank S tile so the
                # VectorE reduce below covers both in a single instruction.
                s2 = psumS.tile([128, 2 * NPAIR], F32, tag="s")
                for h in range(2):
                    dloc = 2 * dp + h
                    sl = slice(dloc * NPAIR, (dloc + 1) * NPAIR)
                    sb = s2[:, h * NPAIR:(h + 1) * NPAIR]
                    # S = Q.P+ (accumulation group stays open)
                    nc.tensor.matmul(sb, lhsT=lhsT,
                                     rhs=psP_sb[:, sl], start=True,
                                     stop=False, skip_group_check=True)
                    # D = Q.P- (separate pool: released after ACT)
                    dt = psumD.tile([128, NPAIR], F32, tag="d")
                    nc.tensor.matmul(dt[:], lhsT=lhsT,
                                     rhs=psM_sb[:, sl], start=True,
                                     stop=True, skip_group_check=True)
                    # A = |D| (fp16, SBUF) on ScalarE — the 2nd PSUM reader
                    a = stage.tile([128, NPAIR], F16)
                    nc.scalar.activation(a[:], dt[:],
                                         mybir.ActivationFunctionType.Abs)
                    # S += I.A  -> S half now holds per-pair maxes
                    nc.tensor.matmul(sb, lhsT=ident_sb[:],
                                     rhs=a[:], start=False, stop=True,
                                     skip_group_check=True)
                col = mg * DOCS_PER_CORE + 2 * dp
                nc.vector.reduce_max(
                    maxcols[:, col:col + 2],
                    s2[:].rearrange("p (h n) -> p h n", h=2),
                    axis=mybir.AxisListType.X)
            if dp == 2:
                # Token-sum part A (docs 0-5 columns) — emitted here so it
                # runs hidden inside the stream; only the small part B
                # remains on the post-stream critical path.
                fin = psumS.tile([QPG, MG * DOCS_PER_CORE], F32, tag="s")
                mc3 = maxcols[:].rearrange("p (mg d) -> p mg d",
                                           d=DOCS_PER_CORE)
                fin3 = fin[:].rearrange("q (mg d) -> q mg d",
                                        d=DOCS_PER_CORE)
                nc.tensor.matmul(fin3[:, :, 0:6], lhsT=ones_sb[:],
                                 rhs=mc3[:, :, 0:6], start=True, stop=True,
                                 skip_group_check=True)

        # Token-sum part B: the last two docs' columns
        nc.tensor.matmul(fin3[:, :, 6:8], lhsT=ones_sb[:],
                         rhs=mc3[:, :, 6:8], start=True, stop=True,
                         skip_group_check=True)
        out_sb = const.tile([QPG, MG * DOCS_PER_CORE], F32)
        nc.vector.tensor_copy(out_sb[:], fin[:])

        # out_sb[q, mg*8 + d] -> out[(mg*4 + q), d]
        out_r = out.rearrange("(mg q) d -> q mg d", q=QPG)
        src = out_sb[:].rearrange("q (mg d) -> q mg d", d=DOCS_PER_CORE)
        nc.sync.dma_start(out_r, src)

    return nc


def _build_direct_module():
    """Exact-fp32 fallback: fp32 matmuls + DVE reduce_max from PSUM."""
    nc = bass.Bass("TRN2", target_bir_lowering=False, debug=False)

    qsT = nc.dram_tensor("qsT", [D, QROWS], F32, kind="ExternalInput").ap()
    psT = nc.dram_tensor("psT", [D, DOCS_PER_CORE * TD], F32,
                         kind="ExternalInput").ap()
    ones = nc.dram_tensor("ones", [128, QPG], F32, kind="ExternalInput").ap()
    out = nc.dram_tensor("out", [NQ, DOCS_PER_CORE], F32,
                         kind="ExternalOutput").ap()

    with tile.TileContext(nc) as tc, ExitStack() as ctx:
        const = ctx.enter_context(tc.tile_pool(name="const", bufs=1))
        psum = ctx.enter_context(tc.tile_pool(name="psum", bufs=3, space="PSUM"))
        psum_fin = ctx.enter_context(
            tc.tile_pool(name="psum_fin", bufs=1, space="PSUM"))

        qsT_sb = const.tile([D, QROWS], F32)
        nc.sync.dma_start(qsT_sb[:], qsT[:])
        ones_sb = const.tile([128, QPG], F32)
        nc.sync.dma_start(ones_sb[:], ones[:])
        psT_sb = const.tile([D, DOCS_PER_CORE * TD], F32)
        for dloc in range(DOCS_PER_CORE):
            sl = slice(dloc * TD, (dloc + 1) * TD)
            nc.sync.dma_start(psT_sb[:, sl], psT[:, sl])

        maxcols = const.tile([128, MG * DOCS_PER_CORE], F32)

        for dloc in range(DOCS_PER_CORE):
            for mg in range(MG):
                pt = psum.tile([128, TD], F32)
                lhsT = qsT_sb[:, mg * 128:(mg + 1) * 128]
                for h in range(TD // 512):
                    nc.tensor.matmul(
                        pt[:, h * 512:(h + 1) * 512],
                        lhsT=lhsT,
                        rhs=psT_sb[:, dloc * TD + h * 512:
                                   dloc * TD + (h + 1) * 512],
                        start=True, stop=True,
                    )
                col = mg * DOCS_PER_CORE + dloc
                nc.vector.reduce_max(
                    maxcols[:, col:col + 1], pt[:],
                    axis=mybir.AxisListType.X)

        fin = psum_fin.tile([QPG, MG * DOCS_PER_CORE], F32)
        nc.tensor.matmul(fin[:], lhsT=ones_sb[:], rhs=maxcols[:],
                         start=True, stop=True)
        out_sb = const.tile([QPG, MG * DOCS_PER_CORE], F32)
        nc.vector.tensor_copy(out_sb[:], fin[:])

        out_r = out.rearrange("(mg q) d -> q mg d", q=QPG)
        src = out_sb[:].rearrange("q (mg d) -> q mg d", d=DOCS_PER_CORE)
        nc.sync.dma_start(out_r, src)

    return nc


_NC_CACHE = {}


def _get_nc(mode=MODE, for_sim=False):
    # The wait-split pass breaks CoreSim's scheduler bookkeeping, so sim
    # uses an unsplit build; hardware needs the split to pass walrus.
    key = (mode, for_sim)
    if key not in _NC_CACHE:
        nc = (_build_pair_module() if mode == "pair"
              else _build_direct_module())
        if not for_sim:
            _split_multi_waits(nc)
        _NC_CACHE[key] = nc
    return _NC_CACHE[key]


def _ones_blockdiag():
    ones = np.zeros((128, QPG), dtype=np.float32)
    for q in range(QPG):
        ones[q * TQ:(q + 1) * TQ, q] = 1.0
    return ones


def _make_in_maps(qs, ps, mode=MODE):
    qs = np.ascontiguousarray(np.asarray(qs), dtype=np.float32)
    ps = np.ascontiguousarray(np.asarray(ps), dtype=np.float32)
    assert qs.shape == (NQ, TQ, D) and ps.shape == (ND, TD, D)
    ones = _ones_blockdiag()

    in_maps = []
    if mode == "pair":
        qsT = np.ascontiguousarray(
            qs.reshape(QROWS, D).T.astype(np.float16))          # [128, 2048]
        pe = ps[:, 0::2, :]
        po = ps[:, 1::2, :]
        pplus = ((pe + po) * 0.5).astype(np.float16)            # [64,512,128]
        pminus = ((pe - po) * 0.5).astype(np.float16)
        ident = np.eye(128, dtype=np.float16)
        for k in range(N_CORES):
            sh = slice(k * DOCS_PER_CORE, (k + 1) * DOCS_PER_CORE)
            pP = np.ascontiguousarray(
                pplus[sh].reshape(DOCS_PER_CORE * NPAIR, D).T)   # [128, 4096]
            pM = np.ascontiguousarray(
                pminus[sh].reshape(DOCS_PER_CORE * NPAIR, D).T)
            in_maps.append({"qsT": qsT, "psP": pP, "psM": pM,
                            "ident": ident, "ones": ones})
    else:
        qsT = np.ascontiguousarray(qs.reshape(QROWS, D).T)      # [128, 2048]
        for k in range(N_CORES):
            shard = ps[k * DOCS_PER_CORE:(k + 1) * DOCS_PER_CORE]
            psTk = np.ascontiguousarray(
                shard.reshape(DOCS_PER_CORE * TD, D).T)
            in_maps.append({"qsT": qsT, "psT": psTk, "ones": ones})
    return in_maps


def _gather(results):
    return np.concatenate(
        [results[k]["out"] for k in range(N_CORES)], axis=1)


def kernel(qs, ps):
    nc = _get_nc()
    in_maps = _make_in_maps(qs, ps)
    res = bass_utils.run_bass_kernel_spmd(
        nc, in_maps, core_ids=list(range(N_CORES)))
    return _gather(res.results)


def kernel_timed(qs, ps, trace_cores=None):
    """Run with NTFF tracing; returns (scores, BassKernelResults)."""
    nc = _get_nc()
    in_maps = _make_in_maps(qs, ps)
    res = bass_utils.run_bass_kernel_spmd(
        nc, in_maps, core_ids=list(range(N_CORES)), trace=True,
        trace_cores=trace_cores)
    return _gather(res.results), res


# revision 37
# speedup vs baseline: 1.1683x; 1.1683x over previous
"""ColBERT MaxSim kernel for 8 Trainium2 NeuronCores.

scores[b, c] = sum_n max_s (qs[b, n, :] . ps[c, s, :])
  qs: (64, 32, 128) f32, ps: (64, 1024, 128) f32 -> scores: (64, 64) f32

Sharding: docs (c) are sharded 8 per core; qs is replicated. Each core
computes its (64, 8) score tile; the host concatenates along c. This puts
only ~2.5 MiB of DMA on each core (vs 33 MiB for batch-sharding) while the
per-core compute volume is identical.

Per-core dataflow (mode "pair", the default):
  - fp32 matmul on TRN2 streams at 2 cyc/col x 2 passes = 4x slower than
    16-bit, so inputs are cast to fp16 on the host (10 mantissa bits;
    measured end-to-end rel err ~2.4e-5 vs the fp32 reference).
  - Doc tokens are combined in PAIRS on the host: P+ = (P_even+P_odd)/2,
    P- = (P_even-P_odd)/2. Using max(a,b) = (a+b)/2 + |a-b|/2:
      PE:  S = Q.P+  (one PSUM bank),  D = Q.P-  (second bank)
      ACT: A = |D|   (pointwise Abs, PSUM->SBUF fp16, the only non-DVE
                      engine that can read PSUM)
      PE:  S += I.A  (accumulating identity matmul, start=False) — S's
                      bank now holds the 512 per-pair maxes
      DVE: reduce_max over [128, 2, 512] (two docs per op) -> maxcols
    This halves the VectorE reduce volume (the kernel's true bottleneck:
    PSUM can only be drained by VectorE/ScalarE at 1 elem/lane/cycle, and
    every reduce op on this silicon runs at 1x regardless of dtype), and
    lands PE/ACT/DVE each at ~650-690 ns per (M-group, doc) tile —
    balanced and ~98% occupied in steady state.
  - Token-sum over each query's 32 rows via one small fp32 matmul with a
    block-diagonal ones matrix: out[4, 128] = onesT.T @ maxcols.

Mode "direct" (env KERNEL_MODE=direct) is the exact-fp32 fallback:
fp32 matmuls + VectorE reduce_max straight from PSUM (~2.6x slower).
"""

import os
import sys
from contextlib import ExitStack

import numpy as np

sys.path.insert(0, "/opt/trn_rl_repo")
sys.path.insert(0, "/opt/trn_rl_repo/concourse")

import bass_rust
import concourse.bass as bass
import concourse.mybir as mybir
import concourse.tile as tile
from concourse import bass_utils

# Problem shape (hardcoded per contract)
N_CORES = 8
NQ, TQ, D = 64, 32, 128          # queries, query tokens, dim
ND, TD = 64, 1024                # docs, doc tokens
DOCS_PER_CORE = ND // N_CORES    # 8
QROWS = NQ * TQ                  # 2048 query-token rows
MG = QROWS // 128                # 16 M-groups of 128 rows
QPG = 128 // TQ                  # 4 queries per M-group
NPAIR = TD // 2                  # 512 token pairs per doc

F32 = mybir.dt.float32
F16 = mybir.dt.float16

MODE = os.environ.get("KERNEL_MODE", "pair")


def _split_multi_waits(nc):
    """This walrus build rejects >1 embedded sync wait per instruction
    ("Too many sync wait commands"). Split extras onto single-wait NoOps
    inserted just before the instruction on the same engine — semantically
    identical (per-engine program order is preserved)."""
    n_split = 0
    for fn in nc.m.functions:
        for blk in fn.blocks:
            out = []
            for ins in blk.instructions:
                si = ins.sync_info
                waits = list(si.on_wait) if si and si.on_wait else []
                if len(waits) > 1:
                    for j, w in enumerate(waits[:-1]):
                        nop = mybir.InstNoOp(
                            name=f"{ins.name}_sw{j}", ins=[], outs=[])
                        nop.engine = ins.engine
                        nop.sync_info = bass_rust.SyncInfo(
                            on_wait=[w], on_update=[])
                        out.append(nop)
                    ins.sync_info = bass_rust.SyncInfo(
                        on_wait=[waits[-1]], on_update=list(si.on_update))
                    n_split += 1
                out.append(ins)
            blk.instructions = out
    return n_split


def _build_pair_module():
    nc = bass.Bass("TRN2", target_bir_lowering=False, debug=False)

    qsT = nc.dram_tensor("qsT", [D, QROWS], F16, kind="ExternalInput").ap()
    psP = nc.dram_tensor("psP", [D, DOCS_PER_CORE * NPAIR], F16,
                         kind="ExternalInput").ap()
    psM = nc.dram_tensor("psM", [D, DOCS_PER_CORE * NPAIR], F16,
                         kind="ExternalInput").ap()
    ident = nc.dram_tensor("ident", [128, 128], F16,
                           kind="ExternalInput").ap()
    ones = nc.dram_tensor("ones", [128, QPG], F32, kind="ExternalInput").ap()
    out = nc.dram_tensor("out", [NQ, DOCS_PER_CORE], F32,
                         kind="ExternalOutput").ap()

    with tile.TileContext(nc) as tc, ExitStack() as ctx:
        const = ctx.enter_context(tc.tile_pool(name="const", bufs=1))
        stage = ctx.enter_context(tc.tile_pool(name="stage", bufs=10))
        psumS = ctx.enter_context(
            tc.tile_pool(name="psumS", bufs=2, space="PSUM"))
        psumD = ctx.enter_context(
            tc.tile_pool(name="psumD", bufs=4, space="PSUM"))

        # DMA issue costs ~650ns per dma_start on a sequencer: consolidate
        # into few chunks and split issues across both HWDGE engines
        # (sync + scalar) so compute starts ASAP. First chunks cover the
        # first two docs only, so the first tiles' deps land quickly.
        qsT_sb = const.tile([D, QROWS], F16)
        psP_sb = const.tile([D, DOCS_PER_CORE * NPAIR], F16)
        psM_sb = const.tile([D, DOCS_PER_CORE * NPAIR], F16)
        ident_sb = const.tile([128, 128], F16)
        ones_sb = const.tile([128, QPG], F32)
        c0 = 2 * NPAIR  # first chunk: docs 0-1
        q0 = 256        # first chunk of qsT: M-groups 0-1 (64 KB)
        nc.sync.dma_start(qsT_sb[:, 0:q0], qsT[:, 0:q0])
        nc.scalar.dma_start(psM_sb[:, 0:c0], psM[:, 0:c0])
        nc.sync.dma_start(psP_sb[:, 0:c0], psP[:, 0:c0])
        # Prefetch the Abs ACT table set (~2.7us TABLE_LOAD + drain) NOW —
        # emitted here it overlaps the initial DMA transfers instead of
        # gating the first real abs (walrus inserts the table load right
        # before the first ACTIVATE in the ScalarE stream).
        warm = stage.tile([1, 2], F16, tag="warm")
        nc.gpsimd.memset(warm[:], 0.0)
        warm2 = stage.tile([1, 2], F16, tag="warm2")
        nc.scalar.activation(warm2[:], warm[:],
                             mybir.ActivationFunctionType.Abs)
        nc.scalar.dma_start(ident_sb[:], ident[:])
        nc.sync.dma_start(qsT_sb[:, q0:], qsT[:, q0:])
        nc.scalar.dma_start(psM_sb[:, c0:], psM[:, c0:])
        nc.sync.dma_start(psP_sb[:, c0:], psP[:, c0:])
        nc.sync.dma_start(ones_sb[:], ones[:])

        # HAM warmup: the PE is otherwise idle from the end of the NEFF
        # preamble (~7.5us) until the first DMA chunks land (~13us), and the
        # HAM clock gate needs ~3.4us of sustained PE activity to lift the
        # throttle from 1.2 to 2.4 GHz. A burst of matmuls on an
        # uninitialized (never-read) tile fills the activity window for
        # free, so the real matmul stream starts warm.
        garbage = const.tile([128, NPAIR], F16)
        nc.gpsimd.memset(garbage[:], 0.0)
        for _ in range(12):
            wt = psumD.tile([128, NPAIR], F32, tag="d")
            nc.tensor.matmul(wt[:], lhsT=garbage[:, 0:128], rhs=garbage[:],
                             start=True, stop=True)
        # Second warmup burst, gated on the first qsT chunk: on cores whose
        # DMA lands late (8-core HBM contention skews arrivals by ~2.5us)
        # the ungated burst ends too early, the PE idles >3.4us and HAM
        # re-throttles right as the real stream starts. These bridge that
        # gap; on fast cores they finish before the ps chunks land anyway.
        for _ in range(6):
            wt = psumD.tile([128, NPAIR], F32, tag="d")
            nc.tensor.matmul(wt[:], lhsT=qsT_sb[:, 0:128],
                             rhs=garbage[:], start=True, stop=True)

        # maxcols[p, mg*8 + dloc] = max over doc dloc's tokens for row p of mg
        maxcols = const.tile([128, MG * DOCS_PER_CORE], F32)

        for dp in range(DOCS_PER_CORE // 2):
            for mg in range(MG):
                lhsT = qsT_sb[:, mg * 128:(mg + 1) * 128]
                # Two docs (2*dp, 2*dp+1) share one 2-bank S tile so the
                # VectorE reduce below covers both in a single instruction.
                s2 = psumS.tile([128, 2 * NPAIR], F32, tag="s")
                for h in range(2):
                    dloc = 2 * dp + h
                    sl = slice(dloc * NPAIR, (dloc + 1) * NPAIR)
                    sb = s2[:, h * NPAIR:(h + 1) * NPAIR]
                    # S = Q.P+ (accumulation group stays open)
                    nc.tensor.matmul(sb, lhsT=lhsT,
                                     rhs=psP_sb[:, sl], start=True,
                                     stop=False, skip_group_check=True)
                    # D = Q.P- (separate pool: released after ACT)
                    dt = psumD.tile([128, NPAIR], F32, tag="d")
                    nc.tensor.matmul(dt[:], lhsT=lhsT,
                                     rhs=psM_sb[:, sl], start=True,
                                     stop=True, skip_group_check=True)
                    # A = |D| (fp16, SBUF) on ScalarE — the 2nd PSUM reader
                    a = stage.tile([128, NPAIR], F16)
                    nc.scalar.activation(a[:], dt[:],
                                         mybir.ActivationFunctionType.Abs)
                    # S += I.A  -> S half now holds per-pair maxes
                    nc.tensor.matmul(sb, lhsT=ident_sb[:],
                                     rhs=a[:], start=False, stop=True,
                                     skip_group_check=True)
                col = mg * DOCS_PER_CORE + 2 * dp
                nc.vector.reduce_max(
                    maxcols[:, col:col + 2],
                    s2[:].rearrange("p (h n) -> p h n", h=2),
                    axis=mybir.AxisListType.X)

        # Token-sum: out[q, col] = sum_p ones[p, q] * maxcols[p, col]
        fin = psumS.tile([QPG, MG * DOCS_PER_CORE], F32, tag="s")
        nc.tensor.matmul(fin[:], lhsT=ones_sb[:], rhs=maxcols[:],
                         start=True, stop=True)
        out_sb = const.tile([QPG, MG * DOCS_PER_CORE], F32)
        nc.vector.tensor_copy(out_sb[:], fin[:])

        # out_sb[q, mg*8 + d] -> out[(mg*4 + q), d]
        out_r = out.rearrange("(mg q) d -> q mg d", q=QPG)
        src = out_sb[:].rearrange("q (mg d) -> q mg d", d=DOCS_PER_CORE)
        nc.sync.dma_start(out_r, src)

    return nc


def _build_direct_module():
    """Exact-fp32 fallback: fp32 matmuls + DVE reduce_max from PSUM."""
    nc = bass.Bass("TRN2", target_bir_lowering=False, debug=False)

    qsT = nc.dram_tensor("qsT", [D, QROWS], F32, kind="ExternalInput").ap()
    psT = nc.dram_tensor("psT", [D, DOCS_PER_CORE * TD], F32,
                         kind="ExternalInput").ap()
    ones = nc.dram_tensor("ones", [128, QPG], F32, kind="ExternalInput").ap()
    out = nc.dram_tensor("out", [NQ, DOCS_PER_CORE], F32,
                         kind="ExternalOutput").ap()

    with tile.TileContext(nc) as tc, ExitStack() as ctx:
        const = ctx.enter_context(tc.tile_pool(name="const", bufs=1))
        psum = ctx.enter_context(tc.tile_pool(name="psum", bufs=3, space="PSUM"))
        psum_fin = ctx.enter_context(
            tc.tile_pool(name="psum_fin", bufs=1, space="PSUM"))

        qsT_sb = const.tile([D, QROWS], F32)
        nc.sync.dma_start(qsT_sb[:], qsT[:])
        ones_sb = const.tile([128, QPG], F32)
        nc.sync.dma_start(ones_sb[:], ones[:])
        psT_sb = const.tile([D, DOCS_PER_CORE * TD], F32)
        for dloc in range(DOCS_PER_CORE):
            sl = slice(dloc * TD, (dloc + 1) * TD)
            nc.sync.dma_start(psT_sb[:, sl], psT[:, sl])

        maxcols = const.tile([128, MG * DOCS_PER_CORE], F32)

        for dloc in range(DOCS_PER_CORE):
            for mg in range(MG):
                pt = psum.tile([128, TD], F32)
                lhsT = qsT_sb[:, mg * 128:(mg + 1) * 128]
                for h in range(TD // 512):
                    nc.tensor.matmul(
                        pt[:, h * 512:(h + 1) * 512],
                        lhsT=lhsT,
                        rhs=psT_sb[:, dloc * TD + h * 512:
                                   dloc * TD + (h + 1) * 512],
                        start=True, stop=True,
                    )
                col = mg * DOCS_PER_CORE + dloc
                nc.vector.reduce_max(
                    maxcols[:, col:col + 1], pt[:],
                    axis=mybir.AxisListType.X)

        fin = psum_fin.tile([QPG, MG * DOCS_PER_CORE], F32)
        nc.tensor.matmul(fin[:], lhsT=ones_sb[:], rhs=maxcols[:],
                         start=True, stop=True)
        out_sb = const.tile([QPG, MG * DOCS_PER_CORE], F32)
        nc.vector.tensor_copy(out_sb[:], fin[:])

        out_r = out.rearrange("(mg q) d -> q mg d", q=QPG)
        src = out_sb[:].rearrange("q (mg d) -> q mg d", d=DOCS_PER_CORE)
        nc.sync.dma_start(out_r, src)

    return nc


_NC_CACHE = {}


def _get_nc(mode=MODE, for_sim=False):
    # The wait-split pass breaks CoreSim's scheduler bookkeeping, so sim
    # uses an unsplit build; hardware needs the split to pass walrus.
    key = (mode, for_sim)
    if key not in _NC_CACHE:
        nc = (_build_pair_module() if mode == "pair"
              else _build_direct_module())
        if not for_sim:
            _split_multi_waits(nc)
        _NC_CACHE[key] = nc
    return _NC_CACHE[key]


def _ones_blockdiag():
    ones = np.zeros((128, QPG), dtype=np.float32)
    for q in range(QPG):
        ones[q * TQ:(q + 1) * TQ, q] = 1.0
    return ones


def _make_in_maps(qs, ps, mode=MODE):
    qs = np.ascontiguousarray(np.asarray(qs), dtype=np.float32)
    ps = np.ascontiguousarray(np.asarray(ps), dtype=np.float32)
    assert qs.shape == (NQ, TQ, D) and ps.shape == (ND, TD, D)
    ones = _ones_blockdiag()

    in_maps = []
    if mode == "pair":
        qsT = np.ascontiguousarray(
            qs.reshape(QROWS, D).T.astype(np.float16))          # [128, 2048]
        pe = ps[:, 0::2, :]
        po = ps[:, 1::2, :]
        pplus = ((pe + po) * 0.5).astype(np.float16)            # [64,512,128]
        pminus = ((pe - po) * 0.5).astype(np.float16)
        ident = np.eye(128, dtype=np.float16)
        for k in range(N_CORES):
            sh = slice(k * DOCS_PER_CORE, (k + 1) * DOCS_PER_CORE)
            pP = np.ascontiguousarray(
                pplus[sh].reshape(DOCS_PER_CORE * NPAIR, D).T)   # [128, 4096]
            pM = np.ascontiguousarray(
                pminus[sh].reshape(DOCS_PER_CORE * NPAIR, D).T)
            in_maps.append({"qsT": qsT, "psP": pP, "psM": pM,
                            "ident": ident, "ones": ones})
    else:
        qsT = np.ascontiguousarray(qs.reshape(QROWS, D).T)      # [128, 2048]
        for k in range(N_CORES):
            shard = ps[k * DOCS_PER_CORE:(k + 1) * DOCS_PER_CORE]
            psTk = np.ascontiguousarray(
                shard.reshape(DOCS_PER_CORE * TD, D).T)
            in_maps.append({"qsT": qsT, "psT": psTk, "ones": ones})
    return in_maps


def _gather(results):
    return np.concatenate(
        [results[k]["out"] for k in range(N_CORES)], axis=1)


def kernel(qs, ps):
    nc = _get_nc()
    in_maps = _make_in_maps(qs, ps)
    res = bass_utils.run_bass_kernel_spmd(
        nc, in_maps, core_ids=list(range(N_CORES)))
    return _gather(res.results)


def kernel_timed(qs, ps, trace_cores=None):
    """Run with NTFF tracing; returns (scores, BassKernelResults)."""
    nc = _get_nc()
    in_maps = _make_in_maps(qs, ps)
    res = bass_utils.run_bass_kernel_spmd(
        nc, in_maps, core_ids=list(range(N_CORES)), trace=True,
        trace_cores=trace_cores)
    return _gather(res.results), res
